# revision 13
# baseline (speedup 1.0000x reference)
"""Multi-head causal attention (B=2, S=2048, E=1024, H=16) on 8 TRN2 cores.

Sharding: 2-way data parallel on batch x 4-way tensor parallel on heads.
Core c handles batch b = c//4 and heads [4g, 4g+4) where g = c%4.
Each core computes q/k/v projections for its 4 heads, causal attention,
and a partial output projection (row-parallel Wo slice); the host sums
the 4 partials per batch and adds bo.

All matmul operands are bf16 (accumulation in fp32 PSUM). Scores are
computed transposed (k on partitions, q on free dim) so the softmax
denominator comes free as an extra ones-row in the P@V matmul, and no
P-tile transposes are needed anywhere.

Scheduling notes (engine-queue order == emission order):
- q-proj is emitted e-outer so the first matmul only needs one weight
  chunk + one xT chunk DMA'd; k/v are emitted bank-major so each PSUM
  bank frees just ahead of its reuse.
- q/k PSUM evictions (bias-add) run on the Activation engine (idle
  during projections); v evictions on DVE.
- Attention pipelines scores one chunk ahead of P@V, with out-proj
  units dripped in as PE filler so the exp (Activation) stage always
  has slack and the PE never idles (sustains max pstate).
- Softmax 1/denom broadcast uses gpsimd partition_broadcast (no DMA).
"""

import sys

sys.path.insert(0, "/opt/trn_rl_repo")

import numpy as np
import ml_dtypes

import concourse.bass as bass  # noqa: F401  (registers engines)
from concourse.ap import AP as _AP


def _free_bcast(src_ap, n):
    """View a [1, F] AP as [1, n, F] with a zero-stride middle dim (DMA replicate)."""
    return _AP(
        src_ap.tensor, src_ap.offset,
        [list(p) for p in src_ap.ap[:1]] + [[0, n]] + [list(p) for p in src_ap.ap[1:]],
    )


import concourse.tile as tile
from concourse import bacc, mybir
from concourse.bass_utils import run_bass_kernel_spmd

B, S, E, H = 2, 2048, 1024, 16
D = E // H            # 64
HPC = H // 4          # 4 heads per core
EC = HPC * D          # 256 = per-core head-dim width
NQT = S // 512        # 4 q-tiles of 512
NKC = S // 128        # 16 k-chunks of 128
NEC = E // 128        # 8 E-chunks of 128

F32 = mybir.dt.float32
BF16 = mybir.dt.bfloat16
EXP = mybir.ActivationFunctionType.Exp
IDENT = mybir.ActivationFunctionType.Identity

# constants blob layout: [128, 961] bf16
#   cols 0:896    staircase mask  M[kk, j] = 1.0 if j >= kk + 384 else 0
#   cols 896:898  ones; cols 898:961 zeros
# (row 0 of the staircase is ones on cols [384:896) — reused as a ones row)
CST_W = 961

# v_sb per k-chunk: [128, 386]
#   h0: cols 0:64 v, 64 ones                 -> lhsT [0:65]   M=65  (sums row 64)
#   h1: col 65 ones, 66:129 zeros, 129:193 v -> lhsT [65:193] M=128 (sums row 0, data rows 64:128)
#   h2: cols 193:257 v, 257 ones             -> lhsT [193:258] M=65
#   h3: col 258 ones, 259:322 zeros, 322:386 v -> lhsT [258:386] M=128
V_W = 386
V_DATA = [0, 129, 193, 322]     # v data col start per local head
V_LHS = [(0, 65), (65, 193), (193, 258), (258, 386)]
V_STATIC = [64, 257]            # col starts of the [1,1,0*63] static blocks


def _build_nc():
    nc = bacc.Bacc("TRN2", target_bir_lowering=False, debug=False, num_devices=8)

    xT = nc.dram_tensor("xT", [E, S], BF16, kind="ExternalInput")
    wq = nc.dram_tensor("wq", [E, EC], BF16, kind="ExternalInput")
    wk = nc.dram_tensor("wk", [E, EC], BF16, kind="ExternalInput")
    wv = nc.dram_tensor("wv", [E, EC], BF16, kind="ExternalInput")
    wo = nc.dram_tensor("wo", [EC, E], BF16, kind="ExternalInput")
    bqd = nc.dram_tensor("bq", [EC], F32, kind="ExternalInput")
    bkd = nc.dram_tensor("bk", [EC], F32, kind="ExternalInput")
    bvd = nc.dram_tensor("bv", [EC], BF16, kind="ExternalInput")
    cst = nc.dram_tensor("cst", [128, CST_W], BF16, kind="ExternalInput")
    out = nc.dram_tensor("out", [S, E], BF16, kind="ExternalOutput")

    from contextlib import ExitStack

    with tile.TileContext(nc) as tc:
        with ExitStack() as stack:
            cpool = stack.enter_context(tc.tile_pool(name="const", bufs=1))
            qkpool = stack.enter_context(tc.tile_pool(name="qkt", bufs=4))
            vpool = stack.enter_context(tc.tile_pool(name="vsb", bufs=NKC))
            proj_stack = ExitStack()
            wpool = proj_stack.enter_context(tc.tile_pool(name="w", bufs=3 * NEC))
            xpool = proj_stack.enter_context(tc.tile_pool(name="xt", bufs=NEC))
            pj_ps = proj_stack.enter_context(tc.tile_pool(name="pj_ps", bufs=8, space="PSUM"))

            # ---- constants + weights + input DMAs ----
            cst_sb = cpool.tile([128, CST_W], BF16, tag="cst")
            nc.sync.dma_start(cst_sb[:], cst[:])
            static_blk = cst_sb[:, 896:961]        # [128,65] = [1,1,0*63]
            ones_row0 = cst_sb[0:1, 384:512]       # [1,128] ones at partition 0

            bq_sb = cpool.tile([128, 2], F32, tag="bq")
            nc.sync.dma_start(bq_sb[:], bqd.ap().rearrange("(b p) -> p b", p=128))
            bk_sb = cpool.tile([128, 2], F32, tag="bk")
            nc.sync.dma_start(bk_sb[:], bkd.ap().rearrange("(b p) -> p b", p=128))
            bv_sb = cpool.tile([1, EC], BF16, tag="bv")
            nc.sync.dma_start(bv_sb[:], bvd.ap().rearrange("(o n) -> o n", o=1))

            w_sb = {}
            for name in ("q", "k", "v"):
                w_sb[name] = [
                    wpool.tile([128, EC], BF16, tag=f"w{name}", name=f"w{name}{e}")
                    for e in range(NEC)
                ]
            # DMA priority: wq chunks and xT stream first (gate the first matmuls)
            nc.sync.dma_start(w_sb["q"][0][:], wq[0:128, :])
            xt_sb = []
            for e in range(NEC):
                t = xpool.tile([128, S], BF16, tag="xt", name=f"xt{e}")
                nc.sync.dma_start(t[:], xT[e * 128:(e + 1) * 128, :])
                xt_sb.append(t)
                if e == 0:
                    for ee in range(1, NEC):
                        nc.sync.dma_start(
                            w_sb["q"][ee][:], wq[ee * 128:(ee + 1) * 128, :])
            for e in range(NEC):
                nc.sync.dma_start(w_sb["k"][e][:], wk[e * 128:(e + 1) * 128, :])
            for e in range(NEC):
                nc.sync.dma_start(w_sb["v"][e][:], wv[e * 128:(e + 1) * 128, :])
            wo_sb = []
            for j in range(2):
                t = cpool.tile([128, E], BF16, tag=f"wo{j}")
                nc.sync.dma_start(t[:], wo[j * 128:(j + 1) * 128, :])
                wo_sb.append(t)

            # preload the exp table set early so it doesn't stall attention
            dummy = cpool.tile([1, 1], F32, tag="dummy")
            nc.scalar.activation(dummy[:], cst_sb[0:1, 0:1], EXP)

            # bv broadcast [128, EC] = ones[1,128].T @ bv[1,EC]  (PE warmup)
            bvb_ps = pj_ps.tile([128, 512], F32, tag="pj", name="bvb")
            nc.tensor.matmul(
                bvb_ps[:, 0:EC], ones_row0, bv_sb[:], start=True, stop=True
            )
            bvb_sb = cpool.tile([128, EC], F32, tag="bvb")
            nc.vector.tensor_copy(bvb_sb[:], bvb_ps[:, 0:EC])

            # ---- q/k projections: qT/kT [pair][128, S] (d on partitions) ----
            # pair p rows: head 2p at partitions 0:64, head 2p+1 at 64:128
            qt_sb = [qkpool.tile([128, S], BF16, tag="qkt", name=f"qt{i}") for i in range(2)]
            kt_sb = [qkpool.tile([128, S], BF16, tag="qkt", name=f"kt{i}") for i in range(2)]

            # q: e-outer (starts as soon as the first chunks land);
            # evictions on the Activation engine (idle here)
            qps = {}
            for pb in range(2):
                for t in range(NQT):
                    qps[pb, t] = pj_ps.tile([128, 512], F32, tag="pj", name=f"qps{pb}_{t}")
            for e in range(NEC):
                for pb in range(2):
                    for t in range(NQT):
                        nc.tensor.matmul(
                            qps[pb, t][:],
                            w_sb["q"][e][:, pb * 128:(pb + 1) * 128],
                            xt_sb[e][:, t * 512:(t + 1) * 512],
                            start=(e == 0),
                            stop=(e == NEC - 1),
                        )
            for pb in range(2):
                for t in range(NQT):
                    nc.scalar.activation(
                        qt_sb[pb][:, t * 512:(t + 1) * 512],
                        qps[pb, t][:], IDENT, bias=bq_sb[:, pb:pb + 1],
                    )

            # k: bank-major (each accumulator's 8 matmuls back-to-back, so
            # bank i is needed right as the ACT eviction of q bank i lands)
            for pb in range(2):
                for t in range(NQT):
                    kps = pj_ps.tile([128, 512], F32, tag="pj", name=f"kps{pb}_{t}")
                    for e in range(NEC):
                        nc.tensor.matmul(
                            kps[:],
                            w_sb["k"][e][:, pb * 128:(pb + 1) * 128],
                            xt_sb[e][:, t * 512:(t + 1) * 512],
                            start=(e == 0),
                            stop=(e == NEC - 1),
                        )
                    nc.scalar.activation(
                        kt_sb[pb][:, t * 512:(t + 1) * 512],
                        kps[:], IDENT, bias=bk_sb[:, pb:pb + 1],
                    )

            # ---- v projection: v_sb [k-chunk][128, V_W] (k on partitions) ----
            v_sb = []
            for m in range(NKC):
                vt = vpool.tile([128, V_W], BF16, tag="vsb")
                for colstart in V_STATIC:
                    nc.vector.tensor_copy(
                        vt[:, colstart:colstart + 65], static_blk
                    )
                vps = pj_ps.tile([128, 512], F32, tag="pj", name=f"vps{m}")
                for e in range(NEC):
                    nc.tensor.matmul(
                        vps[:, 0:EC],
                        xt_sb[e][:, m * 128:(m + 1) * 128],
                        w_sb["v"][e][:],
                        start=(e == 0),
                        stop=(e == NEC - 1),
                    )
                for h in range(HPC):
                    d0 = V_DATA[h]
                    nc.vector.tensor_add(
                        vt[:, d0:d0 + 64],
                        vps[:, h * 64:(h + 1) * 64],
                        bvb_sb[:, h * 64:(h + 1) * 64],
                    )
                v_sb.append(vt)

            # ---- attention, pipelined with out-proj PE filler ----
            proj_stack.close()  # free the projection psum pool + w/x tiles
            apool = stack.enter_context(tc.tile_pool(name="asb", bufs=2 * NQT))
            ppool = stack.enter_context(tc.tile_pool(name="psb", bufs=4))
            rspool = stack.enter_context(tc.tile_pool(name="rs", bufs=4))
            bcpool = stack.enter_context(tc.tile_pool(name="bc", bufs=2))
            arpool = stack.enter_context(tc.tile_pool(name="ar", bufs=2))
            opool = stack.enter_context(tc.tile_pool(name="osb", bufs=4))
            attn_stack = ExitStack()
            qk_ps = attn_stack.enter_context(tc.tile_pool(name="qk_ps", bufs=2, space="PSUM"))
            at_ps = attn_stack.enter_context(tc.tile_pool(name="at_ps", bufs=2, space="PSUM"))
            op_ps = attn_stack.enter_context(tc.tile_pool(name="op_ps", bufs=2, space="PSUM"))
            # per-(pair, q-tile) attn tiles: out-proj units then only depend
            # on their own tile's writes (the pool tracks deps per tile)
            a_sb = [[apool.tile([128, 512], BF16, tag="asb", name=f"a{i}_{t}")
                     for t in range(NQT)] for i in range(2)]

            def oproj_unit(m, n):
                # out-proj unit: out[q,e] = sum_hd A[hd,q] Wo[hd,e]
                mt, mm = divmod(m, 4)
                ops = op_ps.tile([128, 512], F32, tag="op", name="ops")
                for j in range(2):
                    nc.tensor.matmul(
                        ops[:],
                        a_sb[j][mt][:, mm * 128:(mm + 1) * 128],
                        wo_sb[j][:, n * 512:(n + 1) * 512],
                        start=(j == 0), stop=(j == 1),
                    )
                osb = opool.tile([128, 512], BF16, tag="osb", name="osb")
                nc.vector.tensor_copy(osb[:], ops[:])
                nc.sync.dma_start(
                    out[m * 128:(m + 1) * 128, n * 512:(n + 1) * 512], osb[:]
                )

            def attn_section(p, t, backlog):
                nchunks = 4 * (t + 1)
                lhs_e = V_LHS[2 * p]      # even head of the pair
                lhs_o = V_LHS[2 * p + 1]  # odd head
                ape = at_ps.tile([128, 512], F32, tag="at", name="ape")
                apo = at_ps.tile([128, 512], F32, tag="at", name="apo")

                def q0_of(c):
                    d0 = c * 128 - t * 512
                    return max(d0, 0)

                def scores(c):
                    # scoresT [k-chunk, q-tile], both heads; exp; diag mask
                    q0 = q0_of(c)
                    qsl = slice(t * 512 + q0, (t + 1) * 512)
                    qkp = qk_ps.tile([128, 1024], F32, tag="qk", name="qkp")
                    nc.tensor.matmul(
                        qkp[:, q0:512],
                        kt_sb[p][0:64, c * 128:(c + 1) * 128],
                        qt_sb[p][0:64, qsl],
                        start=True, stop=True,
                    )
                    nc.tensor.matmul(
                        qkp[:, 512 + q0:1024],
                        kt_sb[p][64:128, c * 128:(c + 1) * 128],
                        qt_sb[p][64:128, qsl],
                        start=True, stop=True,
                    )
                    psb = ppool.tile([128, 1024], BF16, tag="psb", name="psb")
                    if q0 == 0:
                        nc.scalar.activation(psb[:], qkp[:], EXP)
                    else:
                        nc.scalar.activation(psb[:, q0:512], qkp[:, q0:512], EXP)
                        nc.scalar.activation(
                            psb[:, 512 + q0:1024], qkp[:, 512 + q0:1024], EXP)
                    d0 = c * 128 - t * 512
                    if d0 >= 0:
                        off = 384 - d0
                        for hh in range(2):
                            nc.vector.tensor_mul(
                                psb[:, hh * 512 + q0:(hh + 1) * 512],
                                psb[:, hh * 512 + q0:(hh + 1) * 512],
                                cst_sb[:, off + q0:off + 512],
                            )
                    return psb

                def pv(c, psb):
                    q0 = q0_of(c)
                    first, last = (c == 0), (c == nchunks - 1)
                    nc.tensor.matmul(
                        ape[0:65, q0:512],
                        v_sb[c][:, lhs_e[0]:lhs_e[1]],
                        psb[:, q0:512],
                        start=first, stop=last,
                    )
                    nc.tensor.matmul(
                        apo[:, q0:512],
                        v_sb[c][:, lhs_o[0]:lhs_o[1]],
                        psb[:, 512 + q0:1024],
                        start=first, stop=last,
                    )

                # software pipeline: scores run one chunk ahead of P@V, with
                # out-proj units as extra PE slack for the exp stage; unit
                # consumption is capped so both sections of a tile get filler
                psbs = {0: scores(0)}
                if nchunks > 1:
                    psbs[1] = scores(1)
                quota = 5
                for u in range(min(2, len(backlog))):
                    oproj_unit(*backlog.pop())
                    quota -= 1
                for c in range(nchunks):
                    if c % 2 == 1 and backlog and quota > 0:
                        oproj_unit(*backlog.pop())
                        quota -= 1
                    pv(c, psbs.pop(c))
                    if c + 2 < nchunks:
                        psbs[c + 2] = scores(c + 2)

                # stage raw attn + denom recips to SBUF so the PV PSUM banks
                # free immediately; the broadcast + normalize then run off
                # the critical path (they only gate the NEXT tile's oproj
                # units, via the per-tile a_sb). araw copies go on the scalar
                # engine (slack from the narrow diagonal exps); recips read
                # the denom rows straight from PSUM on DVE.
                ssb = rspool.tile([128, 512], F32, tag="ssb", name="ssb")
                rsf = rspool.tile([128, 512], F32, tag="rsf", name="rsf")
                araw = arpool.tile([128, 512], BF16, tag="ar", name="araw")
                nc.vector.tensor_copy(ssb[64:65, :], ape[64:65, :])
                nc.vector.tensor_copy(ssb[0:1, :], apo[0:1, :])
                nc.scalar.copy(araw[0:64, :], ape[0:64, :])
                nc.scalar.copy(araw[64:128, :], apo[64:128, :])
                # rows 1-63 are garbage; only rows 0 and 64 are read below
                nc.vector.reciprocal_approx_fast(
                    out=rsf[0:65, :], in_=ssb[0:65, :])
                bcs = bcpool.tile([128, 512], F32, tag="bc", name="bcs")
                nc.sync.dma_start(bcs[0:64, :], _free_bcast(rsf[64:65, :], 64))
                nc.sync.dma_start(bcs[64:128, :], _free_bcast(rsf[0:1, :], 64))
                nc.vector.tensor_mul(
                    a_sb[p][t][0:64, :], araw[0:64, :], bcs[0:64, :])
                nc.vector.tensor_mul(
                    a_sb[p][t][64:128, :], araw[64:128, :], bcs[64:128, :])

            # pair-interleaved sections; completed q-tiles' out-proj units are
            # dripped into later sections as PE filler work. Units age one
            # section before use so their a_sb normalization (gated on the
            # recip broadcast DMA) is guaranteed complete.
            backlog = []
            aging = []
            for t in range(NQT):
                attn_section(0, t, backlog)
                backlog.extend(aging)
                aging = []
                attn_section(1, t, backlog)
                aging = [(m, n) for m in range(4 * t, 4 * (t + 1)) for n in range(2)]
            for m, n in backlog + aging:
                oproj_unit(m, n)
            attn_stack.close()

    nc.compile()
    return nc


_NC = None


def _get_nc():
    global _NC
    if _NC is None:
        _NC = _build_nc()
    return _NC


def _constants():
    kk = np.arange(128, dtype=np.int64)[:, None]
    jj = np.arange(896, dtype=np.int64)[None, :]
    cst = np.zeros((128, CST_W), dtype=np.float32)
    cst[:, 0:896] = (jj >= kk + 384).astype(np.float32)
    cst[:, 896] = 1.0
    cst[:, 897] = 1.0
    return cst.astype(ml_dtypes.bfloat16)


def _in_maps(inputs, Wq, bq, Wk, bk, Wv, bv, Wo, bo):
    bf16 = ml_dtypes.bfloat16
    cst = _constants()
    scale = np.float32(1.0 / np.sqrt(D))
    xT = [np.ascontiguousarray(inputs[b].T).astype(bf16) for b in range(B)]

    in_maps = []
    for c in range(8):
        b, g = divmod(c, 4)
        sl = slice(g * EC, (g + 1) * EC)
        in_maps.append({
            "xT": xT[b],
            "wq": (np.ascontiguousarray(Wq[:, sl]) * scale).astype(bf16),
            "bq": (bq[sl] * scale).astype(np.float32),
            "wk": np.ascontiguousarray(Wk[:, sl]).astype(bf16),
            "bk": bk[sl].astype(np.float32),
            "wv": np.ascontiguousarray(Wv[:, sl]).astype(bf16),
            "bv": bv[sl].astype(bf16),
            "wo": np.ascontiguousarray(Wo[sl, :]).astype(bf16),
            "cst": cst,
        })
    return in_maps


def kernel(inputs, Wq, bq, Wk, bk, Wv, bv, Wo, bo):
    inputs = np.asarray(inputs, dtype=np.float32)
    Wq = np.asarray(Wq, dtype=np.float32)
    Wk = np.asarray(Wk, dtype=np.float32)
    Wv = np.asarray(Wv, dtype=np.float32)
    Wo = np.asarray(Wo, dtype=np.float32)
    bq = np.asarray(bq, dtype=np.float32)
    bk = np.asarray(bk, dtype=np.float32)
    bv = np.asarray(bv, dtype=np.float32)
    bo = np.asarray(bo, dtype=np.float32)

    nc = _get_nc()
    in_maps = _in_maps(inputs, Wq, bq, Wk, bk, Wv, bv, Wo, bo)
    res = run_bass_kernel_spmd(nc, in_maps, list(range(8)))
    outs = [np.asarray(r["out"]).astype(np.float32) for r in res.results]
    full = np.empty((B, S, E), dtype=np.float32)
    for b in range(B):
        full[b] = outs[4 * b] + outs[4 * b + 1] + outs[4 * b + 2] + outs[4 * b + 3]
        full[b] += bo
    return full


# revision 20
# speedup vs baseline: 1.2674x; 1.2674x over previous
"""Multi-head causal attention (B=2, S=2048, E=1024, H=16) on 8 TRN2 cores.

Sharding: 2-way data parallel on batch x 4-way tensor parallel on heads.
Core c handles batch b = c//4 and heads [4g, 4g+4) where g = c%4.
Each core computes q/k/v projections for its 4 heads, causal attention,
and a partial output projection (row-parallel Wo slice); the host sums
the 4 partials per batch and adds bo.

All matmul operands are bf16 (accumulation in fp32 PSUM). Scores are
computed transposed (k on partitions, q on free dim) so the softmax
denominator comes free as an extra ones-row in the P@V matmul, and no
P-tile transposes are needed anywhere.

Scheduling notes (engine-queue order == emission order):
- q-proj is emitted e-outer so the first matmul only needs one weight
  chunk + one xT chunk DMA'd; k/v are emitted bank-major so each PSUM
  bank frees just ahead of its reuse.
- q/k PSUM evictions (bias-add) run on the Activation engine (idle
  during projections); v evictions on DVE.
- Attention pipelines scores one chunk ahead of P@V, with out-proj
  units dripped in as PE filler so the exp (Activation) stage always
  has slack and the PE never idles (sustains max pstate).
- Softmax 1/denom broadcast uses gpsimd partition_broadcast (no DMA).
"""

import sys

sys.path.insert(0, "/opt/trn_rl_repo")

import numpy as np
import ml_dtypes

import concourse.bass as bass  # noqa: F401  (registers engines)
from concourse.ap import AP as _AP


def _free_bcast(src_ap, n):
    """View a [1, F] AP as [1, n, F] with a zero-stride middle dim (DMA replicate)."""
    return _AP(
        src_ap.tensor, src_ap.offset,
        [list(p) for p in src_ap.ap[:1]] + [[0, n]] + [list(p) for p in src_ap.ap[1:]],
    )


import concourse.tile as tile
from concourse import bacc, mybir
from concourse.bass_utils import run_bass_kernel_spmd

B, S, E, H = 2, 2048, 1024, 16
D = E // H            # 64
HPC = H // 4          # 4 heads per core
EC = HPC * D          # 256 = per-core head-dim width
NQT = S // 512        # 4 q-tiles of 512
NKC = S // 128        # 16 k-chunks of 128
NEC = E // 128        # 8 E-chunks of 128

F32 = mybir.dt.float32
BF16 = mybir.dt.bfloat16
EXP = mybir.ActivationFunctionType.Exp
IDENT = mybir.ActivationFunctionType.Identity

# constants blob layout: [128, 961] bf16
#   cols 0:896    staircase mask  M[kk, j] = 1.0 if j >= kk + 384 else 0
#   cols 896:898  ones; cols 898:961 zeros
# (row 0 of the staircase is ones on cols [384:896) — reused as a ones row)
CST_W = 961

# v_sb per k-chunk: [128, 386]
#   h0: cols 0:64 v, 64 ones                 -> lhsT [0:65]   M=65  (sums row 64)
#   h1: col 65 ones, 66:129 zeros, 129:193 v -> lhsT [65:193] M=128 (sums row 0, data rows 64:128)
#   h2: cols 193:257 v, 257 ones             -> lhsT [193:258] M=65
#   h3: col 258 ones, 259:322 zeros, 322:386 v -> lhsT [258:386] M=128
V_W = 386
V_DATA = [0, 129, 193, 322]     # v data col start per local head
V_LHS = [(0, 65), (65, 193), (193, 258), (258, 386)]
V_STATIC = [64, 257]            # col starts of the [1,1,0*63] static blocks


def _build_nc():
    nc = bacc.Bacc("TRN2", target_bir_lowering=False, debug=False, num_devices=8)

    xT = nc.dram_tensor("xT", [E, S], BF16, kind="ExternalInput")
    wq = nc.dram_tensor("wq", [E, EC], BF16, kind="ExternalInput")
    wk = nc.dram_tensor("wk", [E, EC], BF16, kind="ExternalInput")
    wv = nc.dram_tensor("wv", [E, EC], BF16, kind="ExternalInput")
    wo = nc.dram_tensor("wo", [EC, E], BF16, kind="ExternalInput")
    bqd = nc.dram_tensor("bq", [EC], F32, kind="ExternalInput")
    bkd = nc.dram_tensor("bk", [EC], F32, kind="ExternalInput")
    bvd = nc.dram_tensor("bv", [EC], BF16, kind="ExternalInput")
    cst = nc.dram_tensor("cst", [128, CST_W], BF16, kind="ExternalInput")
    out = nc.dram_tensor("out", [S, E], BF16, kind="ExternalOutput")

    from contextlib import ExitStack

    with tile.TileContext(nc) as tc:
        with ExitStack() as stack:
            cpool = stack.enter_context(tc.tile_pool(name="const", bufs=1))
            qkpool = stack.enter_context(tc.tile_pool(name="qkt", bufs=4))
            vpool = stack.enter_context(tc.tile_pool(name="vsb", bufs=NKC))
            proj_stack = ExitStack()
            wpool = proj_stack.enter_context(tc.tile_pool(name="w", bufs=3 * NEC))
            xpool = proj_stack.enter_context(tc.tile_pool(name="xt", bufs=NEC))
            pj_ps = proj_stack.enter_context(tc.tile_pool(name="pj_ps", bufs=8, space="PSUM"))

            # ---- constants + weights + input DMAs ----
            cst_sb = cpool.tile([128, CST_W], BF16, tag="cst")
            nc.sync.dma_start(cst_sb[:], cst[:])
            static_blk = cst_sb[:, 896:961]        # [128,65] = [1,1,0*63]
            ones_row0 = cst_sb[0:1, 384:512]       # [1,128] ones at partition 0

            bq_sb = cpool.tile([128, 2], F32, tag="bq")
            nc.sync.dma_start(bq_sb[:], bqd.ap().rearrange("(b p) -> p b", p=128))
            bk_sb = cpool.tile([128, 2], F32, tag="bk")
            nc.sync.dma_start(bk_sb[:], bkd.ap().rearrange("(b p) -> p b", p=128))
            bv_sb = cpool.tile([1, EC], BF16, tag="bv")
            nc.sync.dma_start(bv_sb[:], bvd.ap().rearrange("(o n) -> o n", o=1))

            w_sb = {}
            for name in ("q", "k", "v"):
                w_sb[name] = [
                    wpool.tile([128, EC], BF16, tag=f"w{name}", name=f"w{name}{e}")
                    for e in range(NEC)
                ]
            # DMA priority: wq chunks and xT stream first (gate the first matmuls)
            nc.sync.dma_start(w_sb["q"][0][:], wq[0:128, :])
            xt_sb = []
            for e in range(NEC):
                t = xpool.tile([128, S], BF16, tag="xt", name=f"xt{e}")
                nc.sync.dma_start(t[:], xT[e * 128:(e + 1) * 128, :])
                xt_sb.append(t)
                if e == 0:
                    for ee in range(1, NEC):
                        nc.sync.dma_start(
                            w_sb["q"][ee][:], wq[ee * 128:(ee + 1) * 128, :])
            for e in range(NEC):
                nc.sync.dma_start(w_sb["k"][e][:], wk[e * 128:(e + 1) * 128, :])
            for e in range(NEC):
                nc.sync.dma_start(w_sb["v"][e][:], wv[e * 128:(e + 1) * 128, :])
            wo_sb = []
            for j in range(2):
                t = cpool.tile([128, E], BF16, tag=f"wo{j}")
                nc.sync.dma_start(t[:], wo[j * 128:(j + 1) * 128, :])
                wo_sb.append(t)

            # preload the exp table set early so it doesn't stall attention
            dummy = cpool.tile([1, 1], F32, tag="dummy")
            nc.scalar.activation(dummy[:], cst_sb[0:1, 0:1], EXP)

            # bv broadcast [128, EC] = ones[1,128].T @ bv[1,EC]  (PE warmup)
            bvb_ps = pj_ps.tile([128, 512], F32, tag="pj", name="bvb")
            nc.tensor.matmul(
                bvb_ps[:, 0:EC], ones_row0, bv_sb[:], start=True, stop=True
            )
            bvb_sb = cpool.tile([128, EC], F32, tag="bvb")
            nc.vector.tensor_copy(bvb_sb[:], bvb_ps[:, 0:EC])

            # ---- q/k projections: qT/kT [pair][128, S] (d on partitions) ----
            # pair p rows: head 2p at partitions 0:64, head 2p+1 at 64:128
            qt_sb = [qkpool.tile([128, S], BF16, tag="qkt", name=f"qt{i}") for i in range(2)]
            kt_sb = [qkpool.tile([128, S], BF16, tag="qkt", name=f"kt{i}") for i in range(2)]

            # q: e-outer (starts as soon as the first chunks land);
            # evictions on the Activation engine (idle here)
            qps = {}
            for pb in range(2):
                for t in range(NQT):
                    qps[pb, t] = pj_ps.tile([128, 512], F32, tag="pj", name=f"qps{pb}_{t}")
            for e in range(NEC):
                for pb in range(2):
                    for t in range(NQT):
                        nc.tensor.matmul(
                            qps[pb, t][:],
                            w_sb["q"][e][:, pb * 128:(pb + 1) * 128],
                            xt_sb[e][:, t * 512:(t + 1) * 512],
                            start=(e == 0),
                            stop=(e == NEC - 1),
                        )
            for pb in range(2):
                for t in range(NQT):
                    nc.scalar.activation(
                        qt_sb[pb][:, t * 512:(t + 1) * 512],
                        qps[pb, t][:], IDENT, bias=bq_sb[:, pb:pb + 1],
                    )

            # k: bank-major (each accumulator's 8 matmuls back-to-back, so
            # bank i is needed right as the ACT eviction of q bank i lands)
            for pb in range(2):
                for t in range(NQT):
                    kps = pj_ps.tile([128, 512], F32, tag="pj", name=f"kps{pb}_{t}")
                    for e in range(NEC):
                        nc.tensor.matmul(
                            kps[:],
                            w_sb["k"][e][:, pb * 128:(pb + 1) * 128],
                            xt_sb[e][:, t * 512:(t + 1) * 512],
                            start=(e == 0),
                            stop=(e == NEC - 1),
                        )
                    nc.scalar.activation(
                        kt_sb[pb][:, t * 512:(t + 1) * 512],
                        kps[:], IDENT, bias=bk_sb[:, pb:pb + 1],
                    )

            # ---- v projection: v_sb [k-chunk][128, V_W] (k on partitions) ----
            v_sb = []
            for m in range(NKC):
                vt = vpool.tile([128, V_W], BF16, tag="vsb")
                for colstart in V_STATIC:
                    nc.vector.tensor_copy(
                        vt[:, colstart:colstart + 65], static_blk
                    )
                vps = pj_ps.tile([128, 512], F32, tag="pj", name=f"vps{m}")
                for e in range(NEC):
                    nc.tensor.matmul(
                        vps[:, 0:EC],
                        xt_sb[e][:, m * 128:(m + 1) * 128],
                        w_sb["v"][e][:],
                        start=(e == 0),
                        stop=(e == NEC - 1),
                    )
                for h in range(HPC):
                    d0 = V_DATA[h]
                    nc.vector.tensor_add(
                        vt[:, d0:d0 + 64],
                        vps[:, h * 64:(h + 1) * 64],
                        bvb_sb[:, h * 64:(h + 1) * 64],
                    )
                v_sb.append(vt)

            # ---- attention, pipelined with out-proj PE filler ----
            proj_stack.close()  # free the projection psum pool + w/x tiles
            apool = stack.enter_context(tc.tile_pool(name="asb", bufs=2 * NQT))
            ppool = stack.enter_context(tc.tile_pool(name="psb", bufs=4))
            rspool = stack.enter_context(tc.tile_pool(name="rs", bufs=6))
            arpool = stack.enter_context(tc.tile_pool(name="ar", bufs=2))
            opool = stack.enter_context(tc.tile_pool(name="osb", bufs=4))
            attn_stack = ExitStack()
            qk_ps = attn_stack.enter_context(tc.tile_pool(name="qk_ps", bufs=2, space="PSUM"))
            at_ps = attn_stack.enter_context(tc.tile_pool(name="at_ps", bufs=2, space="PSUM"))
            op_ps = attn_stack.enter_context(tc.tile_pool(name="op_ps", bufs=2, space="PSUM"))
            # per-(pair, q-tile) attn tiles: out-proj units then only depend
            # on their own tile's writes (the pool tracks deps per tile)
            a_sb = [[apool.tile([128, 512], BF16, tag="asb", name=f"a{i}_{t}")
                     for t in range(NQT)] for i in range(2)]

            def oproj_unit(m, n):
                # out-proj unit: out[q,e] = sum_hd A[hd,q] Wo[hd,e]
                mt, mm = divmod(m, 4)
                ops = op_ps.tile([128, 512], F32, tag="op", name="ops")
                for j in range(2):
                    nc.tensor.matmul(
                        ops[:],
                        a_sb[j][mt][:, mm * 128:(mm + 1) * 128],
                        wo_sb[j][:, n * 512:(n + 1) * 512],
                        start=(j == 0), stop=(j == 1),
                    )
                osb = opool.tile([128, 512], BF16, tag="osb", name="osb")
                nc.vector.tensor_copy(osb[:], ops[:])
                nc.sync.dma_start(
                    out[m * 128:(m + 1) * 128, n * 512:(n + 1) * 512], osb[:]
                )

            def finish_norm(p, t, rsf, araw):
                # broadcast 1/denom across partitions with two rank-1
                # matmuls (ones-column lhsT x recip row) into PSUM -- no DMA
                # involved -- then normalize araw into the per-tile a_sb.
                # Emitted early in the NEXT section so the PE never waits on
                # the recip chain.
                bcp = op_ps.tile([128, 512], F32, tag="op", name="bcp")
                nc.tensor.matmul(
                    bcp[0:64, :], cst_sb[64:65, 448:512], rsf[64:65, :],
                    start=True, stop=True, skip_group_check=True,
                )
                nc.tensor.matmul(
                    bcp[64:128, :], cst_sb[0:1, 384:448], rsf[0:1, :],
                    start=True, stop=True, skip_group_check=True,
                )
                nc.vector.tensor_mul(
                    a_sb[p][t][0:64, :], araw[0:64, :], bcp[0:64, :])
                nc.vector.tensor_mul(
                    a_sb[p][t][64:128, :], araw[64:128, :], bcp[64:128, :])

            def attn_section(p, t, backlog, pending_norm):
                nchunks = 4 * (t + 1)
                lhs_e = V_LHS[2 * p]      # even head of the pair
                lhs_o = V_LHS[2 * p + 1]  # odd head
                ape = at_ps.tile([128, 512], F32, tag="at", name="ape")
                apo = at_ps.tile([128, 512], F32, tag="at", name="apo")

                def q0_of(c):
                    d0 = c * 128 - t * 512
                    return max(d0, 0)

                def scores(c):
                    # scoresT [k-chunk, q-tile], both heads; exp; diag mask
                    q0 = q0_of(c)
                    qsl = slice(t * 512 + q0, (t + 1) * 512)
                    qkp = qk_ps.tile([128, 1024], F32, tag="qk", name="qkp")
                    nc.tensor.matmul(
                        qkp[:, q0:512],
                        kt_sb[p][0:64, c * 128:(c + 1) * 128],
                        qt_sb[p][0:64, qsl],
                        start=True, stop=True,
                    )
                    nc.tensor.matmul(
                        qkp[:, 512 + q0:1024],
                        kt_sb[p][64:128, c * 128:(c + 1) * 128],
                        qt_sb[p][64:128, qsl],
                        start=True, stop=True,
                    )
                    psb = ppool.tile([128, 1024], BF16, tag="psb", name="psb")
                    if q0 == 0:
                        nc.scalar.activation(psb[:], qkp[:], EXP)
                    else:
                        nc.scalar.activation(psb[:, q0:512], qkp[:, q0:512], EXP)
                        nc.scalar.activation(
                            psb[:, 512 + q0:1024], qkp[:, 512 + q0:1024], EXP)
                    d0 = c * 128 - t * 512
                    if d0 >= 0:
                        off = 384 - d0
                        for hh in range(2):
                            nc.vector.tensor_mul(
                                psb[:, hh * 512 + q0:(hh + 1) * 512],
                                psb[:, hh * 512 + q0:(hh + 1) * 512],
                                cst_sb[:, off + q0:off + 512],
                            )
                    return psb

                def pv(c, psb):
                    q0 = q0_of(c)
                    first, last = (c == 0), (c == nchunks - 1)
                    nc.tensor.matmul(
                        ape[0:65, q0:512],
                        v_sb[c][:, lhs_e[0]:lhs_e[1]],
                        psb[:, q0:512],
                        start=first, stop=last,
                    )
                    nc.tensor.matmul(
                        apo[:, q0:512],
                        v_sb[c][:, lhs_o[0]:lhs_o[1]],
                        psb[:, 512 + q0:1024],
                        start=first, stop=last,
                    )

                # software pipeline: scores run one chunk ahead of P@V, with
                # out-proj units as extra PE slack for the exp stage; unit
                # consumption is capped so both sections of a tile get filler
                psbs = {0: scores(0)}
                if nchunks > 1:
                    psbs[1] = scores(1)
                if pending_norm:
                    finish_norm(*pending_norm)
                quota = 5
                for u in range(min(2, len(backlog))):
                    oproj_unit(*backlog.pop())
                    quota -= 1
                for c in range(nchunks):
                    if c % 2 == 1 and backlog and quota > 0:
                        oproj_unit(*backlog.pop())
                        quota -= 1
                    pv(c, psbs.pop(c))
                    if c + 2 < nchunks:
                        psbs[c + 2] = scores(c + 2)

                # stage raw attn + denom recips to SBUF so the PV PSUM banks
                # free immediately; the normalize itself is deferred into the
                # next section (finish_norm). araw copies go on the scalar
                # engine (slack from the narrow diagonal exps).
                ssb = rspool.tile([128, 512], F32, tag="ssb", name="ssb")
                rsf = rspool.tile([128, 512], F32, tag="rsf", name="rsf")
                araw = arpool.tile([128, 512], BF16, tag="ar", name="araw")
                nc.vector.tensor_copy(ssb[64:65, :], ape[64:65, :])
                nc.vector.tensor_copy(ssb[0:1, :], apo[0:1, :])
                nc.scalar.copy(araw[0:64, :], ape[0:64, :])
                nc.scalar.copy(araw[64:128, :], apo[64:128, :])
                # rows 1-63 are garbage; only rows 0 and 64 are read below
                nc.vector.reciprocal_approx_fast(
                    out=rsf[0:65, :], in_=ssb[0:65, :])
                rsb = rspool.tile([128, 512], BF16, tag="rsb", name="rsb")
                nc.vector.tensor_copy(rsb[0:65, :], rsf[0:65, :])
                return (p, t, rsb, araw)

            # pair-interleaved sections; completed q-tiles' out-proj units are
            # dripped into later sections as PE filler work. Units age one
            # section before use so their a_sb normalization (gated on the
            # recip broadcast DMA) is guaranteed complete.
            backlog = []
            aging = []
            pend = None
            for t in range(NQT):
                pend = attn_section(0, t, backlog, pend)
                backlog.extend(aging)
                aging = []
                pend = attn_section(1, t, backlog, pend)
                aging = [(m, n) for m in range(4 * t, 4 * (t + 1)) for n in range(2)]
            finish_norm(*pend)
            for m, n in backlog + aging:
                oproj_unit(m, n)
            attn_stack.close()

    nc.compile()
    return nc


_NC = None


def _get_nc():
    global _NC
    if _NC is None:
        _NC = _build_nc()
    return _NC


def _constants():
    kk = np.arange(128, dtype=np.int64)[:, None]
    jj = np.arange(896, dtype=np.int64)[None, :]
    cst = np.zeros((128, CST_W), dtype=np.float32)
    cst[:, 0:896] = (jj >= kk + 384).astype(np.float32)
    cst[:, 896] = 1.0
    cst[:, 897] = 1.0
    return cst.astype(ml_dtypes.bfloat16)


def _in_maps(inputs, Wq, bq, Wk, bk, Wv, bv, Wo, bo):
    bf16 = ml_dtypes.bfloat16
    cst = _constants()
    scale = np.float32(1.0 / np.sqrt(D))
    xT = [np.ascontiguousarray(inputs[b].T).astype(bf16) for b in range(B)]

    in_maps = []
    for c in range(8):
        b, g = divmod(c, 4)
        sl = slice(g * EC, (g + 1) * EC)
        in_maps.append({
            "xT": xT[b],
            "wq": (np.ascontiguousarray(Wq[:, sl]) * scale).astype(bf16),
            "bq": (bq[sl] * scale).astype(np.float32),
            "wk": np.ascontiguousarray(Wk[:, sl]).astype(bf16),
            "bk": bk[sl].astype(np.float32),
            "wv": np.ascontiguousarray(Wv[:, sl]).astype(bf16),
            "bv": bv[sl].astype(bf16),
            "wo": np.ascontiguousarray(Wo[sl, :]).astype(bf16),
            "cst": cst,
        })
    return in_maps


def kernel(inputs, Wq, bq, Wk, bk, Wv, bv, Wo, bo):
    inputs = np.asarray(inputs, dtype=np.float32)
    Wq = np.asarray(Wq, dtype=np.float32)
    Wk = np.asarray(Wk, dtype=np.float32)
    Wv = np.asarray(Wv, dtype=np.float32)
    Wo = np.asarray(Wo, dtype=np.float32)
    bq = np.asarray(bq, dtype=np.float32)
    bk = np.asarray(bk, dtype=np.float32)
    bv = np.asarray(bv, dtype=np.float32)
    bo = np.asarray(bo, dtype=np.float32)

    nc = _get_nc()
    in_maps = _in_maps(inputs, Wq, bq, Wk, bk, Wv, bv, Wo, bo)
    res = run_bass_kernel_spmd(nc, in_maps, list(range(8)))
    outs = [np.asarray(r["out"]).astype(np.float32) for r in res.results]
    full = np.empty((B, S, E), dtype=np.float32)
    for b in range(B):
        full[b] = outs[4 * b] + outs[4 * b + 1] + outs[4 * b + 2] + outs[4 * b + 3]
        full[b] += bo
    return full


# revision 28
# speedup vs baseline: 1.2707x; 1.0026x over previous
"""Multi-head causal attention (B=2, S=2048, E=1024, H=16) on 8 TRN2 cores.

Sharding: 2-way data parallel on batch x 4-way tensor parallel on heads.
Core c handles batch b = c//4 and heads [4g, 4g+4) where g = c%4.
Each core computes q/k/v projections for its 4 heads, causal attention,
and a partial output projection (row-parallel Wo slice); the host sums
the 4 partials per batch and adds bo.

All matmul operands are bf16 (accumulation in fp32 PSUM). Scores are
computed transposed (k on partitions, q on free dim) so the softmax
denominator comes free as an extra ones-row in the P@V matmul, and no
P-tile transposes are needed anywhere.

Scheduling notes (engine-queue order == emission order):
- q-proj is emitted e-outer so the first matmul only needs one weight
  chunk + one xT chunk DMA'd; k/v are emitted bank-major so each PSUM
  bank frees just ahead of its reuse.
- q/k PSUM evictions (bias-add) run on the Activation engine (idle
  during projections); v evictions on DVE.
- Attention pipelines scores one chunk ahead of P@V, with out-proj
  units dripped in as PE filler so the exp (Activation) stage always
  has slack and the PE never idles (sustains max pstate).
- Softmax 1/denom broadcast uses gpsimd partition_broadcast (no DMA).
"""

import sys

sys.path.insert(0, "/opt/trn_rl_repo")

import numpy as np
import ml_dtypes

import concourse.bass as bass  # noqa: F401  (registers engines)
from concourse.ap import AP as _AP


def _free_bcast(src_ap, n):
    """View a [1, F] AP as [1, n, F] with a zero-stride middle dim (DMA replicate)."""
    return _AP(
        src_ap.tensor, src_ap.offset,
        [list(p) for p in src_ap.ap[:1]] + [[0, n]] + [list(p) for p in src_ap.ap[1:]],
    )


import concourse.tile as tile
from concourse import bacc, mybir
from concourse.bass_utils import run_bass_kernel_spmd

B, S, E, H = 2, 2048, 1024, 16
D = E // H            # 64
HPC = H // 4          # 4 heads per core
EC = HPC * D          # 256 = per-core head-dim width
NQT = S // 512        # 4 q-tiles of 512
NKC = S // 128        # 16 k-chunks of 128
NEC = E // 128        # 8 E-chunks of 128

F32 = mybir.dt.float32
BF16 = mybir.dt.bfloat16
EXP = mybir.ActivationFunctionType.Exp
IDENT = mybir.ActivationFunctionType.Identity

# constants blob layout: [128, 833] bf16
#   cols 0:512    staircase mask  M[kk, x] = 1.0 if x >= kk else 0
#                 (diag chunks read [0 : 512-q0]; row 0 doubles as a ones row,
#                  row 64 cols 64:128 as the even-head bcast ones)
#   cols 512:577  static v block [1,1,0*63]
#   cols 577:833  row 0 only: bv (per-core slice)
CST_W = 833

# v_sb per k-chunk: [128, 386]
#   h0: cols 0:64 v, 64 ones                 -> lhsT [0:65]   M=65  (sums row 64)
#   h1: col 65 ones, 66:129 zeros, 129:193 v -> lhsT [65:193] M=128 (sums row 0, data rows 64:128)
#   h2: cols 193:257 v, 257 ones             -> lhsT [193:258] M=65
#   h3: col 258 ones, 259:322 zeros, 322:386 v -> lhsT [258:386] M=128
V_W = 386
V_DATA = [0, 129, 193, 322]     # v data col start per local head
V_LHS = [(0, 65), (65, 193), (193, 258), (258, 386)]
V_STATIC = [64, 257]            # col starts of the [1,1,0*63] static blocks


def _build_nc():
    nc = bacc.Bacc("TRN2", target_bir_lowering=False, debug=False, num_devices=8)

    xT = nc.dram_tensor("xT", [E, S], BF16, kind="ExternalInput")
    wq = nc.dram_tensor("wq", [E, EC], BF16, kind="ExternalInput")
    wk = nc.dram_tensor("wk", [E, EC], BF16, kind="ExternalInput")
    wv = nc.dram_tensor("wv", [E, EC], BF16, kind="ExternalInput")
    wo = nc.dram_tensor("wo", [EC, E], BF16, kind="ExternalInput")
    bqk = nc.dram_tensor("bqk", [128, 4], F32, kind="ExternalInput")
    cst = nc.dram_tensor("cst", [128, CST_W], BF16, kind="ExternalInput")
    out = nc.dram_tensor("out", [S, E], BF16, kind="ExternalOutput")

    from contextlib import ExitStack

    with tile.TileContext(nc) as tc:
        with ExitStack() as stack:
            cpool = stack.enter_context(tc.tile_pool(name="const", bufs=1))
            qkpool = stack.enter_context(tc.tile_pool(name="qkt", bufs=4))
            vpool = stack.enter_context(tc.tile_pool(name="vsb", bufs=NKC))
            proj_stack = ExitStack()
            wpool = proj_stack.enter_context(tc.tile_pool(name="w", bufs=3 * NEC))
            xpool = proj_stack.enter_context(tc.tile_pool(name="xt", bufs=NEC))
            pj_ps = proj_stack.enter_context(tc.tile_pool(name="pj_ps", bufs=8, space="PSUM"))

            # ---- constants + weights + input DMAs ----
            # (each dma_start costs ~650ns of sync-queue dispatch; order and
            # count are tuned so the first q-proj matmul starts earliest)
            cst_sb = cpool.tile([128, CST_W], BF16, tag="cst")
            nc.sync.dma_start(cst_sb[:], cst[:])
            static_blk = cst_sb[:, 512:577]        # [128,65] = [1,1,0*63]
            ones_row0 = cst_sb[0:1, 0:128]         # [1,128] ones at partition 0
            bv_sb = cst_sb[0:1, 577:577 + EC]      # [1,EC] bv at partition 0

            w_sb = {}
            for name in ("q", "k", "v"):
                w_sb[name] = [
                    wpool.tile([128, EC], BF16, tag=f"w{name}", name=f"w{name}{e}")
                    for e in range(NEC)
                ]
            # DMA priority: wq chunks and xT stream first (gate the first matmuls)
            nc.sync.dma_start(w_sb["q"][0][:], wq[0:128, :])
            xt_sb = []
            for e in range(NEC):
                t = xpool.tile([128, S], BF16, tag="xt", name=f"xt{e}")
                nc.sync.dma_start(t[:], xT[e * 128:(e + 1) * 128, :])
                xt_sb.append(t)
                if e == 0:
                    for ee in range(1, NEC):
                        nc.sync.dma_start(
                            w_sb["q"][ee][:], wq[ee * 128:(ee + 1) * 128, :])
            bqk_sb = cpool.tile([128, 4], F32, tag="bqk")
            nc.sync.dma_start(bqk_sb[:], bqk[:])
            bq_sb = bqk_sb[:, 0:2]
            bk_sb = bqk_sb[:, 2:4]
            for e in range(NEC):
                nc.sync.dma_start(w_sb["k"][e][:], wk[e * 128:(e + 1) * 128, :])
            for e in range(NEC):
                nc.sync.dma_start(w_sb["v"][e][:], wv[e * 128:(e + 1) * 128, :])
            wo_sb = []
            for j in range(2):
                t = cpool.tile([128, E], BF16, tag=f"wo{j}")
                nc.sync.dma_start(t[:], wo[j * 128:(j + 1) * 128, :])
                wo_sb.append(t)

            # preload the exp table set early so it doesn't stall attention
            dummy = cpool.tile([1, 1], F32, tag="dummy")
            nc.scalar.activation(dummy[:], cst_sb[0:1, 0:1], EXP)

            # bv broadcast [128, EC] = ones[1,128].T @ bv[1,EC]  (PE warmup)
            bvb_ps = pj_ps.tile([128, 512], F32, tag="pj", name="bvb")
            nc.tensor.matmul(
                bvb_ps[:, 0:EC], ones_row0, bv_sb, start=True, stop=True
            )
            bvb_sb = cpool.tile([128, EC], F32, tag="bvb")
            nc.vector.tensor_copy(bvb_sb[:], bvb_ps[:, 0:EC])

            # ---- q/k projections: qT/kT [pair][128, S] (d on partitions) ----
            # pair p rows: head 2p at partitions 0:64, head 2p+1 at 64:128
            qt_sb = [qkpool.tile([128, S], BF16, tag="qkt", name=f"qt{i}") for i in range(2)]
            kt_sb = [qkpool.tile([128, S], BF16, tag="qkt", name=f"kt{i}") for i in range(2)]

            # q: e-outer (starts as soon as the first chunks land);
            # evictions on the Activation engine (idle here)
            qps = {}
            for pb in range(2):
                for t in range(NQT):
                    qps[pb, t] = pj_ps.tile([128, 512], F32, tag="pj", name=f"qps{pb}_{t}")
            for e in range(NEC):
                for pb in range(2):
                    for t in range(NQT):
                        nc.tensor.matmul(
                            qps[pb, t][:],
                            w_sb["q"][e][:, pb * 128:(pb + 1) * 128],
                            xt_sb[e][:, t * 512:(t + 1) * 512],
                            start=(e == 0),
                            stop=(e == NEC - 1),
                        )
            for pb in range(2):
                for t in range(NQT):
                    nc.scalar.activation(
                        qt_sb[pb][:, t * 512:(t + 1) * 512],
                        qps[pb, t][:], IDENT, bias=bq_sb[:, pb:pb + 1],
                    )

            # k: bank-major (each accumulator's 8 matmuls back-to-back, so
            # bank i is needed right as the ACT eviction of q bank i lands)
            for pb in range(2):
                for t in range(NQT):
                    kps = pj_ps.tile([128, 512], F32, tag="pj", name=f"kps{pb}_{t}")
                    for e in range(NEC):
                        nc.tensor.matmul(
                            kps[:],
                            w_sb["k"][e][:, pb * 128:(pb + 1) * 128],
                            xt_sb[e][:, t * 512:(t + 1) * 512],
                            start=(e == 0),
                            stop=(e == NEC - 1),
                        )
                    nc.scalar.activation(
                        kt_sb[pb][:, t * 512:(t + 1) * 512],
                        kps[:], IDENT, bias=bk_sb[:, pb:pb + 1],
                    )

            # ---- v projection: v_sb [k-chunk][128, V_W] (k on partitions) ----
            v_sb = []
            for m in range(NKC):
                vt = vpool.tile([128, V_W], BF16, tag="vsb")
                for colstart in V_STATIC:
                    nc.vector.tensor_copy(
                        vt[:, colstart:colstart + 65], static_blk
                    )
                vps = pj_ps.tile([128, 512], F32, tag="pj", name=f"vps{m}")
                for e in range(NEC):
                    nc.tensor.matmul(
                        vps[:, 0:EC],
                        xt_sb[e][:, m * 128:(m + 1) * 128],
                        w_sb["v"][e][:],
                        start=(e == 0),
                        stop=(e == NEC - 1),
                    )
                for h in range(HPC):
                    d0 = V_DATA[h]
                    nc.vector.tensor_add(
                        vt[:, d0:d0 + 64],
                        vps[:, h * 64:(h + 1) * 64],
                        bvb_sb[:, h * 64:(h + 1) * 64],
                    )
                v_sb.append(vt)

            # ---- attention, pipelined with out-proj PE filler ----
            proj_stack.close()  # free the projection psum pool + w/x tiles
            apool = stack.enter_context(tc.tile_pool(name="asb", bufs=2 * NQT))
            ppool = stack.enter_context(tc.tile_pool(name="psb", bufs=4))
            rspool = stack.enter_context(tc.tile_pool(name="rs", bufs=6))
            arpool = stack.enter_context(tc.tile_pool(name="ar", bufs=2))
            opool = stack.enter_context(tc.tile_pool(name="osb", bufs=4))
            attn_stack = ExitStack()
            qk_ps = attn_stack.enter_context(tc.tile_pool(name="qk_ps", bufs=2, space="PSUM"))
            at_ps = attn_stack.enter_context(tc.tile_pool(name="at_ps", bufs=2, space="PSUM"))
            op_ps = attn_stack.enter_context(tc.tile_pool(name="op_ps", bufs=2, space="PSUM"))
            # per-(pair, q-tile) attn tiles: out-proj units then only depend
            # on their own tile's writes (the pool tracks deps per tile)
            a_sb = [[apool.tile([128, 512], BF16, tag="asb", name=f"a{i}_{t}")
                     for t in range(NQT)] for i in range(2)]

            def oproj_unit(m, n):
                # out-proj unit: out[q,e] = sum_hd A[hd,q] Wo[hd,e]
                mt, mm = divmod(m, 4)
                ops = op_ps.tile([128, 512], F32, tag="op", name="ops")
                for j in range(2):
                    nc.tensor.matmul(
                        ops[:],
                        a_sb[j][mt][:, mm * 128:(mm + 1) * 128],
                        wo_sb[j][:, n * 512:(n + 1) * 512],
                        start=(j == 0), stop=(j == 1),
                    )
                osb = opool.tile([128, 512], BF16, tag="osb", name="osb")
                nc.vector.tensor_copy(osb[:], ops[:])
                nc.sync.dma_start(
                    out[m * 128:(m + 1) * 128, n * 512:(n + 1) * 512], osb[:]
                )

            def finish_norm(p, t, rsf, araw):
                # broadcast 1/denom across partitions with two rank-1
                # matmuls (ones-column lhsT x recip row) into PSUM -- no DMA
                # involved -- then normalize araw into the per-tile a_sb.
                # Emitted early in the NEXT section so the PE never waits on
                # the recip chain.
                bcp = op_ps.tile([128, 512], F32, tag="op", name="bcp")
                nc.tensor.matmul(
                    bcp[0:64, :], cst_sb[64:65, 64:128], rsf[64:65, :],
                    start=True, stop=True, skip_group_check=True,
                )
                nc.tensor.matmul(
                    bcp[64:128, :], cst_sb[0:1, 0:64], rsf[0:1, :],
                    start=True, stop=True, skip_group_check=True,
                )
                nc.vector.tensor_mul(
                    a_sb[p][t][0:64, :], araw[0:64, :], bcp[0:64, :])
                nc.vector.tensor_mul(
                    a_sb[p][t][64:128, :], araw[64:128, :], bcp[64:128, :])

            def attn_section(p, t, backlog, pending_norm):
                nchunks = 4 * (t + 1)
                lhs_e = V_LHS[2 * p]      # even head of the pair
                lhs_o = V_LHS[2 * p + 1]  # odd head
                ape = at_ps.tile([128, 512], F32, tag="at", name="ape")
                apo = at_ps.tile([128, 512], F32, tag="at", name="apo")

                def q0_of(c):
                    d0 = c * 128 - t * 512
                    return max(d0, 0)

                def scores(c):
                    # scoresT [k-chunk, q-tile], both heads; exp; diag mask
                    q0 = q0_of(c)
                    qsl = slice(t * 512 + q0, (t + 1) * 512)
                    qkp = qk_ps.tile([128, 1024], F32, tag="qk", name="qkp")
                    nc.tensor.matmul(
                        qkp[:, q0:512],
                        kt_sb[p][0:64, c * 128:(c + 1) * 128],
                        qt_sb[p][0:64, qsl],
                        start=True, stop=True,
                    )
                    nc.tensor.matmul(
                        qkp[:, 512 + q0:1024],
                        kt_sb[p][64:128, c * 128:(c + 1) * 128],
                        qt_sb[p][64:128, qsl],
                        start=True, stop=True,
                    )
                    psb = ppool.tile([128, 1024], BF16, tag="psb", name="psb")
                    if q0 <= 256:
                        # one wide exp (cols [512:512+q0] are unread garbage);
                        # fewer activations beats fewer columns here
                        nc.scalar.activation(
                            psb[:, q0:1024], qkp[:, q0:1024], EXP)
                    else:
                        nc.scalar.activation(psb[:, q0:512], qkp[:, q0:512], EXP)
                        nc.scalar.activation(
                            psb[:, 512 + q0:1024], qkp[:, 512 + q0:1024], EXP)
                    d0 = c * 128 - t * 512
                    if d0 >= 0:
                        for hh in range(2):
                            nc.vector.tensor_mul(
                                psb[:, hh * 512 + q0:(hh + 1) * 512],
                                psb[:, hh * 512 + q0:(hh + 1) * 512],
                                cst_sb[:, 0:512 - q0],
                            )
                    return psb

                def pv(c, psb):
                    q0 = q0_of(c)
                    first, last = (c == 0), (c == nchunks - 1)
                    nc.tensor.matmul(
                        ape[0:65, q0:512],
                        v_sb[c][:, lhs_e[0]:lhs_e[1]],
                        psb[:, q0:512],
                        start=first, stop=last,
                    )
                    nc.tensor.matmul(
                        apo[:, q0:512],
                        v_sb[c][:, lhs_o[0]:lhs_o[1]],
                        psb[:, 512 + q0:1024],
                        start=first, stop=last,
                    )

                # software pipeline: scores run one chunk ahead of P@V, with
                # out-proj units as extra PE slack for the exp stage; unit
                # consumption is capped so both sections of a tile get filler
                psbs = {0: scores(0)}
                if nchunks > 1:
                    psbs[1] = scores(1)
                if pending_norm:
                    finish_norm(*pending_norm)
                quota = 6
                for u in range(min(2, len(backlog))):
                    oproj_unit(*backlog.pop())
                    quota -= 1
                for c in range(nchunks):
                    if c % 2 == 1 and backlog and quota > 0:
                        oproj_unit(*backlog.pop())
                        quota -= 1
                    pv(c, psbs.pop(c))
                    if c + 2 < nchunks:
                        psbs[c + 2] = scores(c + 2)

                # stage raw attn + denom recips to SBUF so the PV PSUM banks
                # free immediately; the normalize itself is deferred into the
                # next section (finish_norm). araw copies go on the scalar
                # engine (slack from the narrow diagonal exps).
                ssb = rspool.tile([128, 512], F32, tag="ssb", name="ssb")
                rsf = rspool.tile([128, 512], F32, tag="rsf", name="rsf")
                araw = arpool.tile([128, 512], BF16, tag="ar", name="araw")
                nc.vector.tensor_copy(ssb[64:65, :], ape[64:65, :])
                nc.vector.tensor_copy(ssb[0:1, :], apo[0:1, :])
                nc.vector.tensor_copy(araw[0:64, :], ape[0:64, :])
                nc.vector.tensor_copy(araw[64:128, :], apo[64:128, :])
                # rows 1-63 are garbage; only rows 0 and 64 are read below
                nc.vector.reciprocal_approx_fast(
                    out=rsf[0:65, :], in_=ssb[0:65, :])
                rsb = rspool.tile([128, 512], BF16, tag="rsb", name="rsb")
                nc.vector.tensor_copy(rsb[0:65, :], rsf[0:65, :])
                return (p, t, rsb, araw)

            # pair-interleaved sections; completed q-tiles' out-proj units are
            # dripped into later sections as PE filler work. Units age one
            # section before use so their a_sb normalization (gated on the
            # recip broadcast DMA) is guaranteed complete.
            backlog = []
            aging = []
            pend = None
            for t in range(NQT):
                pend = attn_section(0, t, backlog, pend)
                backlog.extend(aging)
                aging = []
                pend = attn_section(1, t, backlog, pend)
                aging = [(m, n) for m in range(4 * t, 4 * (t + 1)) for n in range(2)]
            finish_norm(*pend)
            for m, n in backlog + aging:
                oproj_unit(m, n)
            attn_stack.close()

    nc.compile()
    return nc


_NC = None


def _get_nc():
    global _NC
    if _NC is None:
        _NC = _build_nc()
    return _NC


def _constants(bv_slice):
    kk = np.arange(128, dtype=np.int64)[:, None]
    jj = np.arange(512, dtype=np.int64)[None, :]
    cst = np.zeros((128, CST_W), dtype=np.float32)
    cst[:, 0:512] = (jj >= kk).astype(np.float32)
    cst[:, 512] = 1.0
    cst[:, 513] = 1.0
    cst[0, 577:577 + EC] = bv_slice
    return cst.astype(ml_dtypes.bfloat16)


def _in_maps(inputs, Wq, bq, Wk, bk, Wv, bv, Wo, bo):
    bf16 = ml_dtypes.bfloat16
    scale = np.float32(1.0 / np.sqrt(D))
    xT = [np.ascontiguousarray(inputs[b].T).astype(bf16) for b in range(B)]

    in_maps = []
    for c in range(8):
        b, g = divmod(c, 4)
        sl = slice(g * EC, (g + 1) * EC)
        bqk = np.empty((128, 4), dtype=np.float32)
        bqk[:, 0] = bq[sl][0:128] * scale
        bqk[:, 1] = bq[sl][128:256] * scale
        bqk[:, 2] = bk[sl][0:128]
        bqk[:, 3] = bk[sl][128:256]
        in_maps.append({
            "xT": xT[b],
            "wq": (np.ascontiguousarray(Wq[:, sl]) * scale).astype(bf16),
            "wk": np.ascontiguousarray(Wk[:, sl]).astype(bf16),
            "wv": np.ascontiguousarray(Wv[:, sl]).astype(bf16),
            "wo": np.ascontiguousarray(Wo[sl, :]).astype(bf16),
            "bqk": bqk,
            "cst": _constants(bv[sl]),
        })
    return in_maps


def kernel(inputs, Wq, bq, Wk, bk, Wv, bv, Wo, bo):
    inputs = np.asarray(inputs, dtype=np.float32)
    Wq = np.asarray(Wq, dtype=np.float32)
    Wk = np.asarray(Wk, dtype=np.float32)
    Wv = np.asarray(Wv, dtype=np.float32)
    Wo = np.asarray(Wo, dtype=np.float32)
    bq = np.asarray(bq, dtype=np.float32)
    bk = np.asarray(bk, dtype=np.float32)
    bv = np.asarray(bv, dtype=np.float32)
    bo = np.asarray(bo, dtype=np.float32)

    nc = _get_nc()
    in_maps = _in_maps(inputs, Wq, bq, Wk, bk, Wv, bv, Wo, bo)
    res = run_bass_kernel_spmd(nc, in_maps, list(range(8)))
    outs = [np.asarray(r["out"]).astype(np.float32) for r in res.results]
    full = np.empty((B, S, E), dtype=np.float32)
    for b in range(B):
        full[b] = outs[4 * b] + outs[4 * b + 1] + outs[4 * b + 2] + outs[4 * b + 3]
        full[b] += bo
    return full
